# revision 5
# baseline (speedup 1.0000x reference)
"""Trainium2 Bass kernel for DenseGINConv (batch of dense graphs).

Reference computation (per graph b):
    agg  = adj[b] @ x[b]                      # [N, F_IN]
    h    = (1 + eps) * x[b] + agg
    h    = relu(h @ W1 + b1) @ W2 + b2        # 2-layer MLP per node
    out  = where(mask[b, :, None], h, 0)

Sharding: pure data parallel — the batch dim B=64 is split 8 ways across the
8 NeuronCores (8 graphs per core); MLP weights and eps are replicated.

Per-core pipeline (all fp32):
  * adj row-blocks [128, 1024] are DMAed contiguously, each 128x128 tile is
    transposed on the TensorEngine (identity matmul), 4 tiles per PSUM bank,
    then copied to SBUF "adjT strips" (alternating Vector/Scalar engines).
  * Aggregation runs in transposed feature space: aggT[f, i] accumulates
    x_tile^T (stationary) @ adjT (streamed, N=512) over the 8 j-tiles.
    The (1+eps)*x term is folded in by streaming (1+eps)*I against the
    diagonal blocks, so no separate x^T is needed.
  * The MLP stays transposed: W1/W2 load naturally as stationary operands,
    biases are per-partition activation biases, relu on the Scalar engine.
  * The result is transposed back per 128-block on the TensorEngine and the
    node mask is applied as a per-partition scalar multiply during PSUM evict.
"""

import numpy as np
from contextlib import ExitStack

B, N, F_IN, F_HID, F_OUT = 64, 1024, 64, 128, 64
N_CORES = 8
BPC = B // N_CORES  # graphs per core
P = 128
NT = N // P  # node tiles per graph

_CACHE = {}


def _build_nc():
    import concourse.mybir as mybir
    import concourse.tile as tile
    from concourse import bacc
    from concourse.masks import make_identity

    fp32 = mybir.dt.float32
    AF = mybir.ActivationFunctionType

    nc = bacc.Bacc(
        "TRN2", target_bir_lowering=False, debug=False, num_devices=N_CORES
    )
    x_d = nc.declare_dram_parameter("x", [BPC, N, F_IN], fp32, isOutput=False)
    adj_d = nc.declare_dram_parameter("adj", [BPC, N, N], fp32, isOutput=False)
    mask_d = nc.declare_dram_parameter(
        "mask", [BPC, N], mybir.dt.uint8, isOutput=False
    )
    w1_d = nc.declare_dram_parameter("W1", [F_IN, F_HID], fp32, isOutput=False)
    b1_d = nc.declare_dram_parameter("b1", [F_HID, 1], fp32, isOutput=False)
    w2_d = nc.declare_dram_parameter("W2", [F_HID, F_OUT], fp32, isOutput=False)
    b2_d = nc.declare_dram_parameter("b2", [F_OUT, 1], fp32, isOutput=False)
    eps_d = nc.declare_dram_parameter("eps", [1, 1], fp32, isOutput=False)
    out_d = nc.declare_dram_parameter("out", [BPC, N, F_OUT], fp32, isOutput=True)

    with tile.TileContext(nc) as tc:
        with ExitStack() as ctx:
            const = ctx.enter_context(tc.tile_pool(name="const", bufs=1))
            ident = const.tile([P, P], fp32)
            make_identity(nc, ident[:])

            w1_sb = const.tile([F_IN, F_HID], fp32)
            nc.sync.dma_start(out=w1_sb[:], in_=w1_d[:])
            w2_sb = const.tile([F_HID, F_OUT], fp32)
            nc.sync.dma_start(out=w2_sb[:], in_=w2_d[:])
            b1_sb = const.tile([F_HID, 1], fp32)
            nc.sync.dma_start(out=b1_sb[:], in_=b1_d[:])
            b2_sb = const.tile([F_OUT, 1], fp32)
            nc.sync.dma_start(out=b2_sb[:], in_=b2_d[:])
            eps_sb = const.tile([1, 1], fp32)
            nc.sync.dma_start(out=eps_sb[:], in_=eps_d[:])

            # c = 1 + eps broadcast to all 128 partitions via a K=1 matmul,
            # then cI = (1+eps) * I for the diagonal fold-in.
            ones_sb = const.tile([1, P], fp32)
            nc.vector.memset(ones_sb[:], 1.0)

            # Working pools
            xp = ctx.enter_context(tc.tile_pool(name="xp", bufs=2))
            maskp = ctx.enter_context(tc.tile_pool(name="maskp", bufs=2))
            rowp = ctx.enter_context(tc.tile_pool(name="rowp", bufs=3))
            adjTp = ctx.enter_context(tc.tile_pool(name="adjTp", bufs=2))
            hp = ctx.enter_context(tc.tile_pool(name="hp", bufs=2))
            a1p = ctx.enter_context(tc.tile_pool(name="a1p", bufs=2))
            z2p = ctx.enter_context(tc.tile_pool(name="z2p", bufs=2))
            outp = ctx.enter_context(tc.tile_pool(name="outp", bufs=2))
            # PSUM: pool_a = 1-bank slots (transposes in, out-transposes out)
            # pool_b = 2-bank slots (aggT / z1 / z2) -> 2*2 + 3*2 = fits 8 banks
            ps_a = ctx.enter_context(tc.tile_pool(name="ps_a", bufs=2, space="PSUM"))
            ps_b = ctx.enter_context(tc.tile_pool(name="ps_b", bufs=3, space="PSUM"))

            c_ps = ps_a.tile([P, 1], fp32, tag="ps_tr")
            nc.tensor.matmul(c_ps[:], ones_sb[:], eps_sb[:], start=True, stop=True)
            c_sb = const.tile([P, 1], fp32)
            nc.scalar.add(c_sb[:], c_ps[:], 1.0)
            ci_sb = const.tile([P, P], fp32)
            nc.vector.tensor_scalar_mul(ci_sb[:], ident[:], c_sb[:, 0:1])

            for b in range(BPC):
                # ---- per-graph inputs
                x_sb = xp.tile([P, NT * F_IN], fp32, tag="x_sb")
                nc.gpsimd.dma_start(
                    out=x_sb[:].rearrange("p (t f) -> p t f", t=NT),
                    in_=x_d[b].rearrange("(t p) f -> p t f", p=P),
                )
                mask_f = maskp.tile([P, NT], fp32, tag="mask_f")
                nc.gpsimd.dma_start(
                    out=mask_f[:],
                    in_=mask_d[b].rearrange("(t p) -> p t", p=P),
                )

                # ---- transpose adj into SBUF strips
                adjT = adjTp.tile([P, NT * N], fp32, tag="adjT")
                adjT3 = adjT[:].rearrange("p (j i) -> p j i", j=NT)
                for it in range(NT):
                    row = rowp.tile([P, N], fp32, tag="row")
                    nc.sync.dma_start(
                        out=row[:], in_=adj_d[b, it * P : (it + 1) * P, :]
                    )
                    for half in range(2):
                        ps_tr = ps_a.tile([P, 4 * P], fp32, tag="ps_tr")
                        for k in range(4):
                            jt = half * 4 + k
                            nc.tensor.transpose(
                                ps_tr[:, k * P : (k + 1) * P],
                                row[:, jt * P : (jt + 1) * P],
                                ident[:],
                            )
                        dest = adjT3[
                            :, half * 4 : (half + 1) * 4, it * P : (it + 1) * P
                        ]
                        src = ps_tr[:].rearrange("p (k i) -> p k i", k=4)
                        if (it + half) % 2 == 0:
                            nc.vector.tensor_copy(dest, src)
                        else:
                            nc.scalar.copy(dest, src)

                # ---- aggregation: aggT[f, i] = sum_j x[j, f] * adjT[j, i]
                agg = ps_b.tile([F_IN, N], fp32, tag="ps_big")
                for jt in range(NT):
                    lhs = x_sb[:, jt * F_IN : (jt + 1) * F_IN]
                    nc.tensor.matmul(
                        agg[:, 0:512],
                        lhs,
                        adjT3[:, jt, 0:512],
                        start=(jt == 0),
                        stop=False,
                    )
                    nc.tensor.matmul(
                        agg[:, 512:1024],
                        lhs,
                        adjT3[:, jt, 512:1024],
                        start=(jt == 0),
                        stop=False,
                    )
                # diagonal fold-in of (1+eps)*x
                for it in range(NT):
                    nc.tensor.matmul(
                        agg[:, it * P : (it + 1) * P],
                        x_sb[:, it * F_IN : (it + 1) * F_IN],
                        ci_sb[:],
                        start=False,
                        stop=(True),
                    )

                hT = hp.tile([F_IN, N], fp32, tag="hT")
                nc.vector.tensor_copy(hT[:, 0:512], agg[:, 0:512])
                nc.scalar.copy(hT[:, 512:1024], agg[:, 512:1024])

                # ---- MLP layer 1 (+relu, +b1)
                z1 = ps_b.tile([F_HID, N], fp32, tag="ps_big")
                nc.tensor.matmul(z1[:, 0:512], w1_sb[:], hT[:, 0:512], start=True, stop=True)
                nc.tensor.matmul(z1[:, 512:1024], w1_sb[:], hT[:, 512:1024], start=True, stop=True)
                a1 = a1p.tile([F_HID, N], fp32, tag="a1")
                nc.scalar.activation(a1[:, 0:512], z1[:, 0:512], AF.Relu, bias=b1_sb[:, 0:1])
                nc.scalar.activation(a1[:, 512:1024], z1[:, 512:1024], AF.Relu, bias=b1_sb[:, 0:1])

                # ---- MLP layer 2 (+b2)
                z2 = ps_b.tile([F_OUT, N], fp32, tag="ps_big")
                nc.tensor.matmul(z2[:, 0:512], w2_sb[:], a1[:, 0:512], start=True, stop=True)
                nc.tensor.matmul(z2[:, 512:1024], w2_sb[:], a1[:, 512:1024], start=True, stop=True)
                z2_sb = z2p.tile([F_OUT, N], fp32, tag="z2_sb")
                nc.scalar.activation(z2_sb[:, 0:512], z2[:, 0:512], AF.Identity, bias=b2_sb[:, 0:1])
                nc.scalar.activation(z2_sb[:, 512:1024], z2[:, 512:1024], AF.Identity, bias=b2_sb[:, 0:1])

                # ---- transpose back to [node, feature] + mask + store
                out_sb = outp.tile([P, NT * F_OUT], fp32, tag="out_sb")
                ps_o = ps_a.tile([P, NT * F_OUT], fp32, tag="ps_tr")
                for it in range(NT):
                    nc.tensor.transpose(
                        ps_o[:, it * F_OUT : (it + 1) * F_OUT],
                        z2_sb[:, it * P : (it + 1) * P],
                        ident[0:F_OUT, 0:F_OUT],
                    )
                    nc.vector.tensor_scalar_mul(
                        out_sb[:, it * F_OUT : (it + 1) * F_OUT],
                        ps_o[:, it * F_OUT : (it + 1) * F_OUT],
                        mask_f[:, it : it + 1],
                    )
                nc.sync.dma_start(
                    out=out_d[b].rearrange("(t p) f -> p t f", p=P),
                    in_=out_sb[:].rearrange("p (t f) -> p t f", t=NT),
                )

    nc.compile()
    return nc


def _get_nc():
    if "nc" not in _CACHE:
        _CACHE["nc"] = _build_nc()
    return _CACHE["nc"]


def _make_in_maps(inputs):
    x = np.asarray(inputs["x"], dtype=np.float32)
    adj = np.asarray(inputs["adj"], dtype=np.float32)
    mask_u8 = np.asarray(inputs["mask"]).astype(np.uint8)
    W1 = np.ascontiguousarray(np.asarray(inputs["W1"], dtype=np.float32))
    b1 = np.asarray(inputs["b1"], dtype=np.float32).reshape(F_HID, 1)
    W2 = np.ascontiguousarray(np.asarray(inputs["W2"], dtype=np.float32))
    b2 = np.asarray(inputs["b2"], dtype=np.float32).reshape(F_OUT, 1)
    eps = np.asarray(inputs["eps"], dtype=np.float32).reshape(1, 1)

    in_maps = []
    for c in range(N_CORES):
        sl = slice(c * BPC, (c + 1) * BPC)
        in_maps.append(
            {
                "x": np.ascontiguousarray(x[sl]),
                "adj": np.ascontiguousarray(adj[sl]),
                "mask": np.ascontiguousarray(mask_u8[sl]),
                "W1": W1,
                "b1": b1,
                "W2": W2,
                "b2": b2,
                "eps": eps,
            }
        )
    return in_maps


def kernel(x, adj, mask, W1, b1, W2, b2, eps):
    from concourse.bass_utils import run_bass_kernel_spmd

    nc = _get_nc()
    in_maps = _make_in_maps(
        dict(x=x, adj=adj, mask=mask, W1=W1, b1=b1, W2=W2, b2=b2, eps=eps)
    )
    res = run_bass_kernel_spmd(nc, in_maps, list(range(N_CORES)))
    out = np.concatenate(
        [res.results[c]["out"] for c in range(N_CORES)], axis=0
    )
    return out


# revision 8
# speedup vs baseline: 13.5582x; 13.5582x over previous
"""Trainium2 Bass kernel for DenseGINConv (batch of dense graphs).

Reference computation (per graph b):
    agg  = adj[b] @ x[b]                      # [N, F_IN]
    h    = (1 + eps) * x[b] + agg
    h    = relu(h @ W1 + b1) @ W2 + b2        # 2-layer MLP per node
    out  = where(mask[b, :, None], h, 0)

Sharding: pure data parallel — the batch dim B=64 is split 8 ways across the
8 NeuronCores (8 graphs per core); MLP weights and eps are replicated.

Per-core pipeline (all fp32):
  * adj row-blocks [128, 1024] are DMAed contiguously, each 128x128 tile is
    transposed on the TensorEngine (identity matmul), 4 tiles per PSUM bank,
    then copied to SBUF "adjT strips" (alternating Vector/Scalar engines).
  * Aggregation runs in transposed feature space: aggT[f, i] accumulates
    x_tile^T (stationary) @ adjT (streamed, N=512) over the 8 j-tiles.
    The (1+eps)*x term is folded in by streaming (1+eps)*I against the
    diagonal blocks, so no separate x^T is needed.
  * The MLP stays transposed: W1/W2 load naturally as stationary operands,
    biases are per-partition activation biases, relu on the Scalar engine.
  * The result is transposed back per 128-block on the TensorEngine and the
    node mask is applied as a per-partition scalar multiply during PSUM evict.
"""

import numpy as np
from contextlib import ExitStack

B, N, F_IN, F_HID, F_OUT = 64, 1024, 64, 128, 64
N_CORES = 8
BPC = B // N_CORES  # graphs per core
P = 128
NT = N // P  # node tiles per graph

_CACHE = {}


def _build_nc(repeat=1):
    import concourse.mybir as mybir
    import concourse.tile as tile
    from concourse import bacc
    from concourse.masks import make_identity

    fp32 = mybir.dt.float32
    AF = mybir.ActivationFunctionType

    nc = bacc.Bacc(
        "TRN2", target_bir_lowering=False, debug=False, num_devices=N_CORES
    )
    x_d = nc.declare_dram_parameter("x", [BPC, N, F_IN], fp32, isOutput=False)
    adj_d = nc.declare_dram_parameter("adj", [BPC, N, N], fp32, isOutput=False)
    mask_d = nc.declare_dram_parameter(
        "mask", [BPC, N], mybir.dt.uint8, isOutput=False
    )
    w1_d = nc.declare_dram_parameter("W1", [F_IN, F_HID], fp32, isOutput=False)
    b1_d = nc.declare_dram_parameter("b1", [F_HID, 1], fp32, isOutput=False)
    w2_d = nc.declare_dram_parameter("W2", [F_HID, F_OUT], fp32, isOutput=False)
    b2_d = nc.declare_dram_parameter("b2", [F_OUT, 1], fp32, isOutput=False)
    eps_d = nc.declare_dram_parameter("eps", [1, 1], fp32, isOutput=False)
    out_d = nc.declare_dram_parameter("out", [BPC, N, F_OUT], fp32, isOutput=True)

    with tile.TileContext(nc) as tc:
        with ExitStack() as ctx:
            const = ctx.enter_context(tc.tile_pool(name="const", bufs=1))
            ident = const.tile([P, P], fp32)
            make_identity(nc, ident[:])

            w1_sb = const.tile([F_IN, F_HID], fp32)
            nc.sync.dma_start(out=w1_sb[:], in_=w1_d[:])
            w2_sb = const.tile([F_HID, F_OUT], fp32)
            nc.sync.dma_start(out=w2_sb[:], in_=w2_d[:])
            b1_sb = const.tile([F_HID, 1], fp32)
            nc.sync.dma_start(out=b1_sb[:], in_=b1_d[:])
            b2_sb = const.tile([F_OUT, 1], fp32)
            nc.sync.dma_start(out=b2_sb[:], in_=b2_d[:])
            eps_sb = const.tile([1, 1], fp32)
            nc.sync.dma_start(out=eps_sb[:], in_=eps_d[:])

            # c = 1 + eps broadcast to all 128 partitions via a K=1 matmul,
            # then cI = (1+eps) * I for the diagonal fold-in.
            ones_sb = const.tile([1, P], fp32)
            nc.vector.memset(ones_sb[:], 1.0)

            # Working pools
            xp = ctx.enter_context(tc.tile_pool(name="xp", bufs=2))
            maskp = ctx.enter_context(tc.tile_pool(name="maskp", bufs=2))
            rowp = ctx.enter_context(tc.tile_pool(name="rowp", bufs=3))
            adjTp = ctx.enter_context(tc.tile_pool(name="adjTp", bufs=2))
            hp = ctx.enter_context(tc.tile_pool(name="hp", bufs=2))
            a1p = ctx.enter_context(tc.tile_pool(name="a1p", bufs=2))
            z2p = ctx.enter_context(tc.tile_pool(name="z2p", bufs=2))
            outp = ctx.enter_context(tc.tile_pool(name="outp", bufs=2))
            # PSUM: pool_a = 1-bank slots (transposes in, out-transposes out)
            # pool_b = 2-bank slots (aggT / z1 / z2) -> 2*2 + 3*2 = fits 8 banks
            ps_a = ctx.enter_context(tc.tile_pool(name="ps_a", bufs=2, space="PSUM"))
            ps_b = ctx.enter_context(tc.tile_pool(name="ps_b", bufs=3, space="PSUM"))

            c_ps = ps_a.tile([P, 1], fp32, tag="ps_tr")
            nc.tensor.matmul(c_ps[:], ones_sb[:], eps_sb[:], start=True, stop=True)
            c_sb = const.tile([P, 1], fp32)
            nc.scalar.add(c_sb[:], c_ps[:], 1.0)
            ci_sb = const.tile([P, P], fp32)
            nc.vector.tensor_scalar_mul(ci_sb[:], ident[:], c_sb[:, 0:1])

            for b in [g for _ in range(repeat) for g in range(BPC)]:
                # ---- per-graph inputs
                x_sb = xp.tile([P, NT * F_IN], fp32, tag="x_sb")
                nc.gpsimd.dma_start(
                    out=x_sb[:].rearrange("p (t f) -> p t f", t=NT),
                    in_=x_d[b].rearrange("(t p) f -> p t f", p=P),
                )
                mask_f = maskp.tile([P, NT], fp32, tag="mask_f")
                nc.gpsimd.dma_start(
                    out=mask_f[:],
                    in_=mask_d[b].rearrange("(t p) -> p t", p=P),
                )

                # ---- transpose adj into SBUF strips
                adjT = adjTp.tile([P, NT * N], fp32, tag="adjT")
                adjT3 = adjT[:].rearrange("p (j i) -> p j i", j=NT)
                for it in range(NT):
                    row = rowp.tile([P, N], fp32, tag="row")
                    nc.sync.dma_start(
                        out=row[:], in_=adj_d[b, it * P : (it + 1) * P, :]
                    )
                    for half in range(2):
                        ps_tr = ps_a.tile([P, 4 * P], fp32, tag="ps_tr")
                        for k in range(4):
                            jt = half * 4 + k
                            nc.tensor.transpose(
                                ps_tr[:, k * P : (k + 1) * P],
                                row[:, jt * P : (jt + 1) * P],
                                ident[:],
                            )
                        dest = adjT3[
                            :, half * 4 : (half + 1) * 4, it * P : (it + 1) * P
                        ]
                        src = ps_tr[:].rearrange("p (k i) -> p k i", k=4)
                        if (it + half) % 2 == 0:
                            nc.vector.tensor_copy(dest, src)
                        else:
                            nc.scalar.copy(dest, src)

                # ---- aggregation: aggT[f, i] = sum_j x[j, f] * adjT[j, i]
                agg = ps_b.tile([F_IN, N], fp32, tag="ps_big")
                for jt in range(NT):
                    lhs = x_sb[:, jt * F_IN : (jt + 1) * F_IN]
                    nc.tensor.matmul(
                        agg[:, 0:512],
                        lhs,
                        adjT3[:, jt, 0:512],
                        start=(jt == 0),
                        stop=False,
                    )
                    nc.tensor.matmul(
                        agg[:, 512:1024],
                        lhs,
                        adjT3[:, jt, 512:1024],
                        start=(jt == 0),
                        stop=False,
                    )
                # diagonal fold-in of (1+eps)*x
                for it in range(NT):
                    nc.tensor.matmul(
                        agg[:, it * P : (it + 1) * P],
                        x_sb[:, it * F_IN : (it + 1) * F_IN],
                        ci_sb[:],
                        start=False,
                        stop=(True),
                    )

                hT = hp.tile([F_IN, N], fp32, tag="hT")
                nc.vector.tensor_copy(hT[:, 0:512], agg[:, 0:512])
                nc.scalar.copy(hT[:, 512:1024], agg[:, 512:1024])

                # ---- MLP layer 1 (+relu, +b1)
                z1 = ps_b.tile([F_HID, N], fp32, tag="ps_big")
                nc.tensor.matmul(z1[:, 0:512], w1_sb[:], hT[:, 0:512], start=True, stop=True)
                nc.tensor.matmul(z1[:, 512:1024], w1_sb[:], hT[:, 512:1024], start=True, stop=True)
                a1 = a1p.tile([F_HID, N], fp32, tag="a1")
                nc.scalar.activation(a1[:, 0:512], z1[:, 0:512], AF.Relu, bias=b1_sb[:, 0:1])
                nc.scalar.activation(a1[:, 512:1024], z1[:, 512:1024], AF.Relu, bias=b1_sb[:, 0:1])

                # ---- MLP layer 2 (+b2)
                z2 = ps_b.tile([F_OUT, N], fp32, tag="ps_big")
                nc.tensor.matmul(z2[:, 0:512], w2_sb[:], a1[:, 0:512], start=True, stop=True)
                nc.tensor.matmul(z2[:, 512:1024], w2_sb[:], a1[:, 512:1024], start=True, stop=True)
                z2_sb = z2p.tile([F_OUT, N], fp32, tag="z2_sb")
                nc.scalar.activation(z2_sb[:, 0:512], z2[:, 0:512], AF.Identity, bias=b2_sb[:, 0:1])
                nc.scalar.activation(z2_sb[:, 512:1024], z2[:, 512:1024], AF.Identity, bias=b2_sb[:, 0:1])

                # ---- transpose back to [node, feature] + mask + store
                out_sb = outp.tile([P, NT * F_OUT], fp32, tag="out_sb")
                ps_o = ps_a.tile([P, NT * F_OUT], fp32, tag="ps_tr")
                for it in range(NT):
                    nc.tensor.transpose(
                        ps_o[:, it * F_OUT : (it + 1) * F_OUT],
                        z2_sb[:, it * P : (it + 1) * P],
                        ident[0:F_OUT, 0:F_OUT],
                    )
                    nc.vector.tensor_scalar_mul(
                        out_sb[:, it * F_OUT : (it + 1) * F_OUT],
                        ps_o[:, it * F_OUT : (it + 1) * F_OUT],
                        mask_f[:, it : it + 1],
                    )
                nc.sync.dma_start(
                    out=out_d[b].rearrange("(t p) f -> p t f", p=P),
                    in_=out_sb[:].rearrange("p (t f) -> p t f", t=NT),
                )

    nc.compile()
    return nc


def _get_nc(repeat=1):
    key = ("nc", repeat)
    if key not in _CACHE:
        _CACHE[key] = _build_nc(repeat)
    return _CACHE[key]


def _make_in_maps(inputs):
    x = np.asarray(inputs["x"], dtype=np.float32)
    adj = np.asarray(inputs["adj"], dtype=np.float32)
    mask_u8 = np.asarray(inputs["mask"]).astype(np.uint8)
    W1 = np.ascontiguousarray(np.asarray(inputs["W1"], dtype=np.float32))
    b1 = np.asarray(inputs["b1"], dtype=np.float32).reshape(F_HID, 1)
    W2 = np.ascontiguousarray(np.asarray(inputs["W2"], dtype=np.float32))
    b2 = np.asarray(inputs["b2"], dtype=np.float32).reshape(F_OUT, 1)
    eps = np.asarray(inputs["eps"], dtype=np.float32).reshape(1, 1)

    in_maps = []
    for c in range(N_CORES):
        sl = slice(c * BPC, (c + 1) * BPC)
        in_maps.append(
            {
                "x": np.ascontiguousarray(x[sl]),
                "adj": np.ascontiguousarray(adj[sl]),
                "mask": np.ascontiguousarray(mask_u8[sl]),
                "W1": W1,
                "b1": b1,
                "W2": W2,
                "b2": b2,
                "eps": eps,
            }
        )
    return in_maps


def kernel(x, adj, mask, W1, b1, W2, b2, eps):
    from concourse.bass_utils import run_bass_kernel_spmd

    nc = _get_nc()
    in_maps = _make_in_maps(
        dict(x=x, adj=adj, mask=mask, W1=W1, b1=b1, W2=W2, b2=b2, eps=eps)
    )
    res = run_bass_kernel_spmd(nc, in_maps, list(range(N_CORES)))
    out = np.concatenate(
        [res.results[c]["out"] for c in range(N_CORES)], axis=0
    )
    return out
